# revision 19
# baseline (speedup 1.0000x reference)
"""Trainium2 Bass kernel for nn_Cross_attention_3 (sparse_attention).

Sharding: spatial over the 10368 unfold patches.  The x-side (img) gives
core k patches [1296k, 1296k+1296) -- one D-block of 9 rows, so the
fold/output stays local.  The y-side (fea -> pooled) gives core k
patches [1280k, 1280k+1408): aligned to the 128-patch pooling bins (11
whole bins per core, neighbours overlap by one bin), so every pooling
bin is computed wholly on one core and the only collective is a 90KB
AllGather of partial pooled bins, fully overlapped with the x-side.

The two MLP linears collapse into one 81x81 matrix; the conv bias rides
as an 82nd contraction row written once per pass.  The 1x1x1 conv uses
the patch data as the matmul's stationary operand so its output lands
directly in (patch-element, channel) layout.  PSUM is evicted in 2-bank
[81,1024] strides; evictions (conv copy, MLP LeakyReLU via
max(0.2x,x), attention copy) are load-balanced across DVE, ACT and
GpSimd.  Attention matmuls for ring r are interleaved into ring r+1's
conv/MLP chunk stream to keep the PE densely fed (p-state ramp) and to
spread eviction load.
"""

import os
import sys

import numpy as np

try:
    import ml_dtypes
except ImportError:
    ml_dtypes = None

try:
    import concourse.bacc as _  # noqa: F401
except ImportError:  # container default path
    sys.path.insert(0, "/opt/trn_rl_repo")

import concourse.bacc as bacc
import concourse.mybir as mybir
from concourse.bass_utils import run_bass_kernel_spmd
from concourse.tile import TileContext

P = 9
P2 = 81
C = 64
D = 72
H = W = 108
HW = H * W
NCORES = 8

# x-side: exact shard, 1296 patches = 648 slot pairs per core
LX = 1296
LPX = LX // 2          # 648
RING = 216             # attention ring, in pairs
NRING = LPX // RING    # 3
NLPB = 24              # pass-B chunk, in pairs
NCHB = RING // NLPB    # 9 chunks per ring
NATT = C // 2          # att tiles (2 channels each) per ring

# y-side: bin-aligned shard with overlap, 1408 patches = 704 pairs
LY = 1408
LPY = LY // 2          # 704
NLPA = 32              # pass-A chunk, in pairs
NCHA = LPY // NLPA     # 22
NBIN = 11              # local pooling bins per core (128 patches each)
NU = 22                # 32-patch units per slot (704/32)

F32 = mybir.dt.float32
BF16 = mybir.dt.bfloat16

_cache = {}


def _build_nc():
    nc = bacc.Bacc(None, target_bir_lowering=False, debug=False)
    xp_d = nc.declare_dram_parameter("xp", [128, LPX, P2], BF16, isOutput=False)
    yp_d = nc.declare_dram_parameter("yp", [128, LPY, P2], BF16, isOutput=False)
    wi_d = nc.declare_dram_parameter("wi", [128, 128], BF16, isOutput=False)
    wf_d = nc.declare_dram_parameter("wf", [128, 128], BF16, isOutput=False)
    wm_d = nc.declare_dram_parameter("wm", [82, P2], BF16, isOutput=False)
    wmy_d = nc.declare_dram_parameter("wmy", [82, P2], BF16, isOutput=False)
    bi_d = nc.declare_dram_parameter("bi", [1, NLPA * 128], BF16, isOutput=False)
    bf_d = nc.declare_dram_parameter("bf", [1, NLPA * 128], BF16, isOutput=False)
    out_d = nc.declare_dram_parameter("out", [P2, C, LX], BF16, isOutput=True)

    gath_d = nc.dram_tensor("gath", [NCORES, P2, NBIN, C], BF16,
                            addr_space="Shared")

    MUL = mybir.AluOpType.mult
    MAX = mybir.AluOpType.max
    ADD = mybir.AluOpType.add

    def evict(eng, dst, src, lrelu=False):
        if lrelu:
            assert eng == "a"
            nc.scalar.activation(dst, src, mybir.ActivationFunctionType.Prelu,
                                 alpha=0.2)
        elif eng == "a":
            nc.scalar.copy(dst, src)
        else:
            e = {"v": nc.vector, "g": nc.gpsimd}[eng]
            e.tensor_copy(dst, src)

    with nc.allow_low_precision("bf16 compute pipeline"), TileContext(nc) as tc:
        with (
            tc.tile_pool(name="const", bufs=1) as constp,
            tc.tile_pool(name="stage", bufs=2) as stagep,
            tc.tile_pool(name="feap", bufs=2) as feap,
            tc.tile_pool(name="treep", bufs=1) as treep,
            tc.tile_pool(name="imgp", bufs=2) as imgp,
            tc.tile_pool(name="evp", bufs=3) as evp,
            tc.tile_pool(name="ps", bufs=2, space="PSUM") as psp,
            tc.tile_pool(name="dram", bufs=1, space="DRAM") as dramp,
        ):
            wi_sb = constp.tile([128, 128], BF16, tag="wi")
            wf_sb = constp.tile([128, 128], BF16, tag="wf")
            wm_sb = constp.tile([82, P2], BF16, tag="wm")
            wmy_sb = constp.tile([82, P2], BF16, tag="wmy")
            ms0 = constp.tile([82, NLPA, 128], BF16, tag="ms0")
            ms1 = constp.tile([82, NLPA, 128], BF16, tag="ms1")
            h32 = constp.tile([P2, 2, NU, C], BF16, tag="h32")
            b1t = constp.tile([P2, NU, C], BF16, tag="b1t")
            part = constp.tile([P2, NBIN, C], BF16, tag="part")
            pooled = constp.tile([P2, C, P2], BF16, tag="pooled")
            stg = constp.tile([P2, NCORES, NBIN, C], BF16, tag="stg")
            part_dram = dramp.tile([P2, NBIN, C], BF16)

            nc.sync.dma_start(out=wi_sb[:, :], in_=wi_d[:, :])
            nc.sync.dma_start(out=wf_sb[:, :], in_=wf_d[:, :])
            nc.sync.dma_start(out=wm_sb[:, :], in_=wm_d[:, :])
            nc.sync.dma_start(out=wmy_sb[:, :], in_=wmy_d[:, :])
            # conv-bias contraction rows, written once per pass
            nc.sync.dma_start(
                out=ms0[81:82, :, :].rearrange("p a b -> p (a b)"), in_=bf_d[:, :]
            )
            nc.sync.dma_start(
                out=ms1[81:82, :, :].rearrange("p a b -> p (a b)"), in_=bf_d[:, :]
            )

            def conv_chunk(st_src, lp0, nlp, w_sb, ms, engs, att_cb=None):
                """stage DMA + conv matmuls + evicts into ms rows 0..80.
                att_cb(t) is called between conv tiles to interleave
                attention tiles into the PE stream."""
                st = stagep.tile([128, NLPA, P2], BF16, tag="st")
                nc.sync.dma_start(
                    out=st[:, 0:nlp, :], in_=st_src[:, lp0:lp0 + nlp, :]
                )
                for t in range(nlp // 4):
                    cv = psp.tile([P2, 512], F32, tag="cv")
                    for j in range(4):
                        nc.tensor.matmul(
                            cv[0:P2, 128 * j:128 * (j + 1)],
                            st[:, 4 * t + j, :], w_sb[:, :],
                            start=True, stop=True,
                        )
                    d = ms[0:P2, 4 * t:4 * t + 4, :].rearrange("p a b -> p (a b)")
                    evict(engs[t], d, cv[0:P2, :])
                    if att_cb is not None:
                        att_cb(t)

            def mlp_tiles(ms, nlp, dst, dst_lp0, wide, w_sb, att_cb=None,
                          dstB=None):
                """MLP matmuls + ACT Prelu evicts into dst at pair offset.
                wide: 8-pair [81,1024] tiles on "at" (no attention active)
                vs 4-pair [81,512] tiles on "ml" (att-active rings).
                dstB: per-slot split (dst=slot0, dstB=slot1, each [.., 64])
                so the attention rhs column stride is 128B, not 256B."""
                flat = ms[:, :, :].rearrange("p a b -> p (a b)")
                if wide:
                    for t in range(nlp // 8):
                        mp = psp.tile([P2, 1024], F32, tag="at")
                        for g in range(2):
                            nc.tensor.matmul(
                                mp[0:P2, 512 * g:512 * (g + 1)],
                                w_sb[:, :],
                                flat[:, 1024 * t + 512 * g:
                                     1024 * t + 512 * (g + 1)],
                                start=True, stop=True,
                            )
                        l0 = dst_lp0 + 8 * t
                        if dstB is None:
                            dq = dst[0:P2, l0:l0 + 8,
                                     :].rearrange("p a b -> p (a b)")
                            evict("a", dq, mp[0:P2, :], lrelu=True)
                        else:
                            sv = mp[0:P2, :].rearrange(
                                "p (l s) -> p l s", s=128)
                            evict("a", dst[0:P2, l0:l0 + 8, :],
                                  sv[:, :, 0:64], lrelu=True)
                            evict("a", dstB[0:P2, l0:l0 + 8, :],
                                  sv[:, :, 64:128], lrelu=True)
                else:
                    for t in range(nlp // 4):
                        mp = psp.tile([P2, 512], F32, tag="ml")
                        nc.tensor.matmul(
                            mp[0:P2, 0:512], w_sb[:, :],
                            flat[:, 512 * t:512 * (t + 1)],
                            start=True, stop=True,
                        )
                        l0 = dst_lp0 + 4 * t
                        sv = mp[0:P2, :].rearrange("p (l s) -> p l s", s=128)
                        evict("a", dst[0:P2, l0:l0 + 4, :],
                              sv[:, :, 0:64], lrelu=True)
                        evict("a", dstB[0:P2, l0:l0 + 4, :],
                              sv[:, :, 64:128], lrelu=True)
                        if att_cb is not None:
                            att_cb(100 + t)

            # ---------------- pass A: fea (y) + local pooled bins -----------
            for ch in range(NCHA):
                ms = ms0 if ch % 2 == 0 else ms1
                # conv evicts: DVE, ACT, ACT, GP; mlp: ACT, GP, GP + 1 ACT
                cengs = (["v", "v", "a", "v", "v", "v", "a", "v"]
                         if ch % 2 == 0 else
                         ["v", "v", "a", "v", "v", "v", "v", "v"])
                conv_chunk(yp_d, NLPA * ch, NLPA, wf_sb, ms, cengs)
                fea = feap.tile([P2, NLPA, 128], BF16, tag="fea")
                mlp_tiles(ms, NLPA, fea, 0, wide=True, w_sb=wmy_sb)
                # pairwise tree over lp: 32 -> 16 -> 8 -> 4 -> 2 -> 1
                # (bf16 2x DVE); 32-patch sums land in h32
                t1 = treep.tile([P2, 16, 128], BF16, tag="t1")
                f2 = fea[0:P2, :, :].rearrange("p (a two) b -> p a two b", two=2)
                nc.vector.tensor_tensor(
                    t1[0:P2, :, :], f2[:, :, 0, :], f2[:, :, 1, :], ADD)
                t2 = treep.tile([P2, 8, 128], BF16, tag="t2")
                t1s = t1[0:P2, :, :].rearrange("p (a two) b -> p a two b", two=2)
                nc.gpsimd.tensor_tensor(
                    t2[0:P2, :, :], t1s[:, :, 0, :], t1s[:, :, 1, :], ADD)
                t3 = treep.tile([P2, 4, 128], BF16, tag="t3")
                t2s = t2[0:P2, :, :].rearrange("p (a two) b -> p a two b", two=2)
                nc.gpsimd.tensor_tensor(
                    t3[0:P2, :, :], t2s[:, :, 0, :], t2s[:, :, 1, :], ADD)
                t4 = treep.tile([P2, 2, 128], BF16, tag="t4")
                t3s = t3[0:P2, :, :].rearrange("p (a two) b -> p a two b", two=2)
                nc.gpsimd.tensor_tensor(
                    t4[0:P2, :, :], t3s[:, :, 0, :], t3s[:, :, 1, :], ADD)
                nc.gpsimd.tensor_tensor(
                    h32[0:P2, :, ch, :],
                    t4[0:P2, 0, :].rearrange("p (s c) -> p s c", s=2),
                    t4[0:P2, 1, :].rearrange("p (s c) -> p s c", s=2),
                    ADD)

            # bin tree: 44 unit sums (t = 22*slot + u are consecutive global
            # 32-patch units) -> 22 -> 11 bins of 128 patches
            hf = h32[0:P2, :, :, :].rearrange("p s u c -> p (s u) c")
            hp = hf.rearrange("p (a two) c -> p a two c", two=2)
            nc.gpsimd.tensor_tensor(
                b1t[0:P2, :, :], hp[:, :, 0, :], hp[:, :, 1, :], ADD)
            b1p = b1t[0:P2, :, :].rearrange("p (a two) c -> p a two c", two=2)
            nc.gpsimd.tensor_tensor(
                part[0:P2, :, :], b1p[:, :, 0, :], b1p[:, :, 1, :], ADD)
            nc.sync.dma_start(out=part_dram[:, :, :], in_=part[:, :, :])
            nc.gpsimd.collective_compute(
                "AllGather",
                mybir.AluOpType.bypass,
                replica_groups=[list(range(NCORES))],
                ins=[part_dram[:, :, :]],
                outs=[gath_d[:, :, :, :]],
            )

            # rewrite bias rows for pass B
            nc.sync.dma_start(
                out=ms0[81:82, :, :].rearrange("p a b -> p (a b)"), in_=bi_d[:, :]
            )
            nc.sync.dma_start(
                out=ms1[81:82, :, :].rearrange("p a b -> p (a b)"), in_=bi_d[:, :]
            )

            # ---------------- pass B: img (x) + attention -------------------
            def att_tiles(img, r_att, i0, n, dma_engs):
                lp0 = RING * r_att
                imgA, imgB = img
                for k in range(n):
                    i = i0 + k
                    ap = psp.tile([P2, 1024], F32, tag="at")
                    for g in range(2):
                        c = 2 * i + g
                        for s, im in ((0, imgA), (1, imgB)):
                            nc.tensor.matmul(
                                ap[0:P2, 512 * g + RING * s:
                                   512 * g + RING * (s + 1)],
                                pooled[:, c:c + 1, :],
                                im[0:P2, :, c:c + 1].rearrange(
                                    "p l c -> p c l"),
                                start=True, stop=True,
                            )
                    ev = evp.tile([P2, 2, 2, RING], BF16, tag="ev")
                    src = ap[0:P2, :].rearrange(
                        "p (g x) -> p g x", g=2)[:, :, 0:2 * RING].rearrange(
                        "p g (s l) -> p g s l", s=2)
                    # split the evict across both engines to release the
                    # PSUM tile twice as fast
                    evict("v", ev[0:P2, 0, :, :], src[:, 0, :, :])
                    evict("a", ev[0:P2, 1, :, :], src[:, 1, :, :])
                    dst = out_d[0:P2, 2 * i:2 * i + 2, :].rearrange(
                        "p c (s l) -> p c s l", s=2)[:, :, :, lp0:lp0 + RING]
                    if dma_engs[k] == "g":
                        nc.gpsimd.dma_start(out=dst, in_=ev[0:P2, :, :, :])
                    else:
                        nc.sync.dma_start(out=dst, in_=ev[0:P2, :, :, :])

            imgs = []
            msc = 0
            for r in range(NRING):
                imgA = imgp.tile([P2, RING, 64], BF16, tag="imA")
                imgB = imgp.tile([P2, RING, 64], BF16, tag="imB")
                img = (imgA, imgB)
                imgs.append(img)
                for chb in range(NCHB):
                    ms = ms0 if msc % 2 == 0 else ms1
                    msc += 1
                    att_cb = None
                    if r >= 1 and chb < 8:
                        slots = {2: 0, 4: 1, 101: 2, 103: 3}
                        def att_cb(t, _img=imgs[r - 1], _r=r - 1, _chb=chb):
                            k = slots.get(t)
                            if k is not None:
                                att_tiles(_img, _r, 4 * _chb + k, 1,
                                          ["s"])
                    cengs = ["v", "v", "a", "v", "v", "v"]
                    conv_chunk(xp_d, RING * r + NLPB * chb, NLPB, wi_sb, ms,
                               cengs, att_cb)
                    mlp_tiles(ms, NLPB, img[0], NLPB * chb, wide=(r == 0),
                              w_sb=wm_sb, att_cb=att_cb, dstB=img[1])
                if r == 0:
                    # pooled assembly (1/128 pre-folded into wmy on host;
                    # pure copies on the otherwise idle GpSimd)
                    nc.sync.dma_start(
                        out=stg[:, :, :, :],
                        in_=gath_d[:, :, :, :].rearrange("k p b c -> p k b c"),
                    )
                    nc.vector.tensor_copy(
                        pooled[0:P2, :, 0:80].rearrange(
                            "p c (k b) -> p k b c", k=8),
                        stg[0:P2, :, 0:10, :],
                    )
                    nc.vector.tensor_copy(
                        pooled[0:P2, :, 80:81].rearrange("p c j -> p j c"),
                        stg[0:P2, 7:8, 10:11, :].rearrange(
                            "p k b c -> p (k b) c"),
                    )
            # trailing attention for the last ring
            dma_engs = ["g" if i % 4 == 1 else "s" for i in range(NATT)]
            att_tiles(imgs[2], 2, 0, NATT, dma_engs)
    nc.compile()
    return nc


def _host_prep(x, y, w_img, b_img, w_fea, b_fea, w1, w2):
    f32 = np.float32
    bf16 = ml_dtypes.bfloat16
    weff = (w2.astype(np.float64) @ w1.astype(np.float64))  # (81, 81)
    wm = np.concatenate([weff.T, weff.sum(axis=1)[None, :]], axis=0)
    wmy = (wm / 128.0).astype(f32).astype(bf16)
    wm = wm.astype(f32).astype(bf16)

    def pairw(w):
        blk = np.zeros((128, 128), dtype=f32)
        blk[0:64, 0:64] = w.T
        blk[64:128, 64:128] = w.T
        return blk.astype(bf16)

    wi = pairw(w_img.astype(f32))
    wf = pairw(w_fea.astype(f32))
    bi = np.tile(np.concatenate([b_img, b_img]).astype(f32), NLPA)[None, :]
    bf_ = np.tile(np.concatenate([b_fea, b_fea]).astype(f32), NLPA)[None, :]
    bi = bi.astype(bf16)
    bf_ = bf_.astype(bf16)

    def unfold(t):  # (1, 64, 72, 108, 108) -> (C, 10368, 81) patch matrix
        u = np.ascontiguousarray(
            t.reshape(C, D // P, P, HW // P, P).transpose(0, 1, 3, 2, 4)
        ).reshape(C, (D // P) * (HW // P), P2)
        return u

    def pack(u, l0, lhalf):  # global patches [l0, l0+2*lhalf) -> [128, lhalf, 81]
        v = u[:, l0:l0 + 2 * lhalf, :].reshape(C, 2, lhalf, P2)
        v = v.transpose(1, 0, 2, 3).reshape(128, lhalf, P2)
        return np.ascontiguousarray(v).astype(ml_dtypes.bfloat16)

    ux = unfold(np.asarray(x, dtype=f32))
    uy = unfold(np.asarray(y, dtype=f32))
    shared = {"wi": wi, "wf": wf, "wm": wm, "wmy": wmy, "bi": bi, "bf": bf_}
    maps = []
    for k in range(NCORES):
        maps.append(dict(
            shared,
            xp=pack(ux, LX * k, LPX),
            yp=pack(uy, (LY - 128) * k, LPY),
        ))
    return maps


def kernel(x, y, w_img, b_img, w_fea, b_fea, w1, w2):
    if "nc" not in _cache:
        _cache["nc"] = _build_nc()
    nc = _cache["nc"]
    in_maps = _host_prep(x, y, w_img, b_img, w_fea, b_fea, w1, w2)
    trace = bool(os.environ.get("KERNEL_TRACE"))
    res = run_bass_kernel_spmd(
        nc, in_maps, list(range(NCORES)), trace=trace
    )
    _cache["last_result"] = res
    out = np.empty((1, C, D, H, W), dtype=np.float32)
    ov = out.reshape(C, D, HW)
    for k in range(NCORES):
        # out_d is (81, 64, 1296) with l = 648*slot + lp (already global l)
        att = res.results[k]["out"].astype(np.float32).transpose(1, 2, 0)
        blk = att.reshape(C, LX, P, P).transpose(0, 2, 1, 3).reshape(C, P, HW)
        ov[:, P * k:P * (k + 1), :] = blk
    return out


# revision 21
# speedup vs baseline: 1.0798x; 1.0798x over previous
"""Trainium2 Bass kernel for nn_Cross_attention_3 (sparse_attention).

Sharding: spatial over the 10368 unfold patches.  The x-side (img) gives
core k patches [1296k, 1296k+1296) -- one D-block of 9 rows, so the
fold/output stays local.  The y-side (fea -> pooled) gives core k
patches [1280k, 1280k+1408): aligned to the 128-patch pooling bins (11
whole bins per core, neighbours overlap by one bin), so every pooling
bin is computed wholly on one core and the only collective is a 90KB
AllGather of partial pooled bins, fully overlapped with the x-side.

The two MLP linears collapse into one 81x81 matrix; the conv bias rides
as an 82nd contraction row written once per pass.  The 1x1x1 conv uses
the patch data as the matmul's stationary operand so its output lands
directly in (patch-element, channel) layout.  PSUM is evicted in 2-bank
[81,1024] strides; evictions (conv copy, MLP LeakyReLU via
max(0.2x,x), attention copy) are load-balanced across DVE, ACT and
GpSimd.  Attention matmuls for ring r are interleaved into ring r+1's
conv/MLP chunk stream to keep the PE densely fed (p-state ramp) and to
spread eviction load.
"""

import os
import sys

import numpy as np

try:
    import ml_dtypes
except ImportError:
    ml_dtypes = None

try:
    import concourse.bacc as _  # noqa: F401
except ImportError:  # container default path
    sys.path.insert(0, "/opt/trn_rl_repo")

import concourse.bacc as bacc
import concourse.mybir as mybir
from concourse.bass_utils import run_bass_kernel_spmd
from concourse.tile import TileContext

P = 9
P2 = 81
C = 64
D = 72
H = W = 108
HW = H * W
NCORES = 8

# x-side: exact shard, 1296 patches = 648 slot pairs per core
LX = 1296
LPX = LX // 2          # 648
RING = 216             # attention ring, in pairs
NRING = LPX // RING    # 3
NLPB = 24              # pass-B chunk, in pairs
NCHB = RING // NLPB    # 9 chunks per ring
NATT = C // 2          # att tiles (2 channels each) per ring

# y-side: bin-aligned shard with overlap, 1408 patches = 704 pairs
LY = 1408
LPY = LY // 2          # 704
NLPA = 32              # pass-A chunk, in pairs
NCHA = LPY // NLPA     # 22
NBIN = 11              # local pooling bins per core (128 patches each)
NU = 22                # 32-patch units per slot (704/32)

F32 = mybir.dt.float32
BF16 = mybir.dt.bfloat16

_cache = {}


def _build_nc():
    nc = bacc.Bacc(None, target_bir_lowering=False, debug=False)
    xp_d = nc.declare_dram_parameter("xp", [128, LPX, P2], BF16, isOutput=False)
    yp_d = nc.declare_dram_parameter("yp", [128, LPY, P2], BF16, isOutput=False)
    wi_d = nc.declare_dram_parameter("wi", [128, 128], BF16, isOutput=False)
    wf_d = nc.declare_dram_parameter("wf", [128, 128], BF16, isOutput=False)
    wm_d = nc.declare_dram_parameter("wm", [82, P2], BF16, isOutput=False)
    wmy_d = nc.declare_dram_parameter("wmy", [82, P2], BF16, isOutput=False)
    bi_d = nc.declare_dram_parameter("bi", [1, NLPA * 128], BF16, isOutput=False)
    bf_d = nc.declare_dram_parameter("bf", [1, NLPA * 128], BF16, isOutput=False)
    out_d = nc.declare_dram_parameter("out", [P2, C, LX], BF16, isOutput=True)

    gath_d = nc.dram_tensor("gath", [NCORES, P2, NBIN, C], BF16,
                            addr_space="Shared")

    MUL = mybir.AluOpType.mult
    MAX = mybir.AluOpType.max
    ADD = mybir.AluOpType.add

    def evict(eng, dst, src, lrelu=False):
        if lrelu:
            assert eng == "a"
            nc.scalar.activation(dst, src, mybir.ActivationFunctionType.Prelu,
                                 alpha=0.2)
        elif eng == "a":
            nc.scalar.copy(dst, src)
        else:
            e = {"v": nc.vector, "g": nc.gpsimd}[eng]
            e.tensor_copy(dst, src)

    with nc.allow_low_precision("bf16 compute pipeline"), TileContext(nc) as tc:
        with (
            tc.tile_pool(name="const", bufs=1) as constp,
            tc.tile_pool(name="stage", bufs=2) as stagep,
            tc.tile_pool(name="feap", bufs=2) as feap,
            tc.tile_pool(name="treep", bufs=1) as treep,
            tc.tile_pool(name="imgp", bufs=2) as imgp,
            tc.tile_pool(name="evp", bufs=3) as evp,
            tc.tile_pool(name="ps", bufs=2, space="PSUM") as psp,
            tc.tile_pool(name="dram", bufs=1, space="DRAM") as dramp,
        ):
            wi_sb = constp.tile([128, 128], BF16, tag="wi")
            wf_sb = constp.tile([128, 128], BF16, tag="wf")
            wm_sb = constp.tile([82, P2], BF16, tag="wm")
            wmy_sb = constp.tile([82, P2], BF16, tag="wmy")
            ms0 = constp.tile([82, NLPA, 128], BF16, tag="ms0")
            ms1 = constp.tile([82, NLPA, 128], BF16, tag="ms1")
            h32 = constp.tile([P2, 2, NU, C], BF16, tag="h32")
            b1t = constp.tile([P2, NU, C], BF16, tag="b1t")
            part = constp.tile([P2, NBIN, C], BF16, tag="part")
            pooled = constp.tile([P2, C, P2], BF16, tag="pooled")
            stg = constp.tile([P2, NCORES, NBIN, C], BF16, tag="stg")
            part_dram = dramp.tile([P2, NBIN, C], BF16)

            nc.sync.dma_start(out=wi_sb[:, :], in_=wi_d[:, :])
            nc.sync.dma_start(out=wf_sb[:, :], in_=wf_d[:, :])
            nc.sync.dma_start(out=wm_sb[:, :], in_=wm_d[:, :])
            nc.sync.dma_start(out=wmy_sb[:, :], in_=wmy_d[:, :])
            # conv-bias contraction rows, written once per pass
            nc.sync.dma_start(
                out=ms0[81:82, :, :].rearrange("p a b -> p (a b)"), in_=bf_d[:, :]
            )
            nc.sync.dma_start(
                out=ms1[81:82, :, :].rearrange("p a b -> p (a b)"), in_=bf_d[:, :]
            )

            def conv_chunk(st_src, lp0, nlp, w_sb, ms, engs, att_cb=None,
                           alt=False):
                """stage DMA + conv matmuls + evicts into ms rows 0..80.
                att_cb(t) is called between conv tiles to interleave
                attention tiles into the PE stream.  alt: rotate conv tiles
                over both cv and the otherwise-idle ml tag (effective PSUM
                depth 4) when no MLP narrow tiles are active."""
                st = stagep.tile([128, NLPA, P2], BF16, tag="st")
                nc.sync.dma_start(
                    out=st[:, 0:nlp, :], in_=st_src[:, lp0:lp0 + nlp, :]
                )
                for t in range(nlp // 4):
                    tag = "ml" if (alt and t % 2 == 1) else "cv"
                    cv = psp.tile([P2, 512], F32, tag=tag)
                    for j in range(4):
                        nc.tensor.matmul(
                            cv[0:P2, 128 * j:128 * (j + 1)],
                            st[:, 4 * t + j, :], w_sb[:, :],
                            start=True, stop=True,
                        )
                    d = ms[0:P2, 4 * t:4 * t + 4, :].rearrange("p a b -> p (a b)")
                    evict(engs[t], d, cv[0:P2, :])
                    if att_cb is not None:
                        att_cb(t)

            def mlp_tiles(ms, nlp, dst, dst_lp0, wide, w_sb, att_cb=None):
                """MLP matmuls + ACT Prelu evicts into dst at pair offset.
                wide: 8-pair [81,1024] tiles on "at" (no attention active)
                vs 4-pair [81,512] tiles on "ml" (att-active rings)."""
                flat = ms[:, :, :].rearrange("p a b -> p (a b)")
                cmajor = dst.shape[1] == 128
                if wide:
                    for t in range(nlp // 8):
                        mp = psp.tile([P2, 1024], F32, tag="at")
                        for g in range(2):
                            nc.tensor.matmul(
                                mp[0:P2, 512 * g:512 * (g + 1)],
                                w_sb[:, :],
                                flat[:, 1024 * t + 512 * g:
                                     1024 * t + 512 * (g + 1)],
                                start=True, stop=True,
                            )
                        if cmajor:
                            dq = dst[0:P2, :, dst_lp0 + 8 * t:
                                     dst_lp0 + 8 * t + 8].rearrange(
                                "p c l -> p l c")
                            sq = mp[0:P2, :].rearrange(
                                "p (l c) -> p l c", c=128)
                        else:
                            dq = dst[0:P2, dst_lp0 + 8 * t:dst_lp0 + 8 * t + 8,
                                     :].rearrange("p a b -> p (a b)")
                            sq = mp[0:P2, :]
                        evict("a", dq, sq, lrelu=True)
                else:
                    for t in range(nlp // 4):
                        mp = psp.tile([P2, 512], F32, tag="ml")
                        nc.tensor.matmul(
                            mp[0:P2, 0:512], w_sb[:, :],
                            flat[:, 512 * t:512 * (t + 1)],
                            start=True, stop=True,
                        )
                        if cmajor:
                            dq = dst[0:P2, :, dst_lp0 + 4 * t:
                                     dst_lp0 + 4 * t + 4].rearrange(
                                "p c l -> p l c")
                            sq = mp[0:P2, :].rearrange(
                                "p (l c) -> p l c", c=128)
                        else:
                            dq = dst[0:P2, dst_lp0 + 4 * t:dst_lp0 + 4 * t + 4,
                                     :].rearrange("p a b -> p (a b)")
                            sq = mp[0:P2, :]
                        evict("a", dq, sq, lrelu=True)
                        if att_cb is not None:
                            att_cb(100 + t)

            # ---------------- pass A: fea (y) + local pooled bins -----------
            for ch in range(NCHA):
                ms = ms0 if ch % 2 == 0 else ms1
                # conv evicts: DVE, ACT, ACT, GP; mlp: ACT, GP, GP + 1 ACT
                cengs = (["v", "v", "a", "v", "v", "v", "a", "v"]
                         if ch % 2 == 0 else
                         ["v", "v", "a", "v", "v", "v", "v", "v"])
                conv_chunk(yp_d, NLPA * ch, NLPA, wf_sb, ms, cengs,
                           alt=True)
                fea = feap.tile([P2, NLPA, 128], BF16, tag="fea")
                mlp_tiles(ms, NLPA, fea, 0, wide=True, w_sb=wmy_sb)
                # pairwise tree over lp: 32 -> 16 -> 8 -> 4 -> 2 -> 1
                # (bf16 2x DVE); 32-patch sums land in h32
                t1 = treep.tile([P2, 16, 128], BF16, tag="t1")
                f2 = fea[0:P2, :, :].rearrange("p (a two) b -> p a two b", two=2)
                nc.vector.tensor_tensor(
                    t1[0:P2, :, :], f2[:, :, 0, :], f2[:, :, 1, :], ADD)
                t2 = treep.tile([P2, 8, 128], BF16, tag="t2")
                t1s = t1[0:P2, :, :].rearrange("p (a two) b -> p a two b", two=2)
                nc.gpsimd.tensor_tensor(
                    t2[0:P2, :, :], t1s[:, :, 0, :], t1s[:, :, 1, :], ADD)
                t3 = treep.tile([P2, 4, 128], BF16, tag="t3")
                t2s = t2[0:P2, :, :].rearrange("p (a two) b -> p a two b", two=2)
                nc.gpsimd.tensor_tensor(
                    t3[0:P2, :, :], t2s[:, :, 0, :], t2s[:, :, 1, :], ADD)
                t4 = treep.tile([P2, 2, 128], BF16, tag="t4")
                t3s = t3[0:P2, :, :].rearrange("p (a two) b -> p a two b", two=2)
                nc.gpsimd.tensor_tensor(
                    t4[0:P2, :, :], t3s[:, :, 0, :], t3s[:, :, 1, :], ADD)
                nc.gpsimd.tensor_tensor(
                    h32[0:P2, :, ch, :],
                    t4[0:P2, 0, :].rearrange("p (s c) -> p s c", s=2),
                    t4[0:P2, 1, :].rearrange("p (s c) -> p s c", s=2),
                    ADD)

            # bin tree: 44 unit sums (t = 22*slot + u are consecutive global
            # 32-patch units) -> 22 -> 11 bins of 128 patches
            hf = h32[0:P2, :, :, :].rearrange("p s u c -> p (s u) c")
            hp = hf.rearrange("p (a two) c -> p a two c", two=2)
            nc.gpsimd.tensor_tensor(
                b1t[0:P2, :, :], hp[:, :, 0, :], hp[:, :, 1, :], ADD)
            b1p = b1t[0:P2, :, :].rearrange("p (a two) c -> p a two c", two=2)
            nc.gpsimd.tensor_tensor(
                part[0:P2, :, :], b1p[:, :, 0, :], b1p[:, :, 1, :], ADD)
            nc.sync.dma_start(out=part_dram[:, :, :], in_=part[:, :, :])
            nc.gpsimd.collective_compute(
                "AllGather",
                mybir.AluOpType.bypass,
                replica_groups=[list(range(NCORES))],
                ins=[part_dram[:, :, :]],
                outs=[gath_d[:, :, :, :]],
            )

            # rewrite bias rows for pass B
            nc.sync.dma_start(
                out=ms0[81:82, :, :].rearrange("p a b -> p (a b)"), in_=bi_d[:, :]
            )
            nc.sync.dma_start(
                out=ms1[81:82, :, :].rearrange("p a b -> p (a b)"), in_=bi_d[:, :]
            )

            # ---------------- pass B: img (x) + attention -------------------
            def att_tiles(img, r_att, i0, n, dma_engs):
                lp0 = RING * r_att
                rhsv = img[0:P2, :, :].rearrange("p l (s c) -> p c s l", s=2)
                for k in range(n):
                    i = i0 + k
                    ap = psp.tile([P2, 1024], F32, tag="at")
                    for g in range(2):
                        c = 2 * i + g
                        nc.tensor.matmul(
                            ap[0:P2, 512 * g:512 * g + 2 * RING],
                            pooled[:, c:c + 1, :], rhsv[:, c:c + 1, :, :],
                            start=True, stop=True,
                        )
                    ev = evp.tile([P2, 2, 2, RING], BF16, tag="ev")
                    src = ap[0:P2, :].rearrange(
                        "p (g x) -> p g x", g=2)[:, :, 0:2 * RING].rearrange(
                        "p g (s l) -> p g s l", s=2)
                    # split the evict across both engines to release the
                    # PSUM tile twice as fast
                    evict("v", ev[0:P2, 0, :, :], src[:, 0, :, :])
                    evict("a", ev[0:P2, 1, :, :], src[:, 1, :, :])
                    dst = out_d[0:P2, 2 * i:2 * i + 2, :].rearrange(
                        "p c (s l) -> p c s l", s=2)[:, :, :, lp0:lp0 + RING]
                    if dma_engs[k] == "g":
                        nc.gpsimd.dma_start(out=dst, in_=ev[0:P2, :, :, :])
                    else:
                        nc.sync.dma_start(out=dst, in_=ev[0:P2, :, :, :])

            imgs = []
            msc = 0
            for r in range(NRING):
                img = imgp.tile([P2, RING, 128], BF16, tag="img")
                imgs.append(img)
                for chb in range(NCHB):
                    ms = ms0 if msc % 2 == 0 else ms1
                    msc += 1
                    att_cb = None
                    if r >= 1 and chb < 8:
                        slots = {2: 0, 4: 1, 101: 2, 103: 3}
                        def att_cb(t, _img=imgs[r - 1], _r=r - 1, _chb=chb):
                            k = slots.get(t)
                            if k is not None:
                                att_tiles(_img, _r, 4 * _chb + k, 1,
                                          ["s"])
                    cengs = ["v", "v", "a", "v", "v", "v"]
                    conv_chunk(xp_d, RING * r + NLPB * chb, NLPB, wi_sb, ms,
                               cengs, att_cb, alt=(r == 0))
                    mlp_tiles(ms, NLPB, img, NLPB * chb, wide=(r == 0),
                              w_sb=wm_sb, att_cb=att_cb)
                if r == 0:
                    # pooled assembly (1/128 pre-folded into wmy on host;
                    # pure copies on the otherwise idle GpSimd)
                    nc.sync.dma_start(
                        out=stg[:, :, :, :],
                        in_=gath_d[:, :, :, :].rearrange("k p b c -> p k b c"),
                    )
                    nc.vector.tensor_copy(
                        pooled[0:P2, :, 0:80].rearrange(
                            "p c (k b) -> p k b c", k=8),
                        stg[0:P2, :, 0:10, :],
                    )
                    nc.vector.tensor_copy(
                        pooled[0:P2, :, 80:81].rearrange("p c j -> p j c"),
                        stg[0:P2, 7:8, 10:11, :].rearrange(
                            "p k b c -> p (k b) c"),
                    )
            # trailing attention for the last ring
            dma_engs = ["g" if i % 4 == 1 else "s" for i in range(NATT)]
            att_tiles(imgs[2], 2, 0, NATT, dma_engs)
    nc.compile()
    return nc


def _host_prep(x, y, w_img, b_img, w_fea, b_fea, w1, w2):
    f32 = np.float32
    bf16 = ml_dtypes.bfloat16
    weff = (w2.astype(np.float64) @ w1.astype(np.float64))  # (81, 81)
    wm = np.concatenate([weff.T, weff.sum(axis=1)[None, :]], axis=0)
    wmy = (wm / 128.0).astype(f32).astype(bf16)
    wm = wm.astype(f32).astype(bf16)

    def pairw(w):
        blk = np.zeros((128, 128), dtype=f32)
        blk[0:64, 0:64] = w.T
        blk[64:128, 64:128] = w.T
        return blk.astype(bf16)

    wi = pairw(w_img.astype(f32))
    wf = pairw(w_fea.astype(f32))
    bi = np.tile(np.concatenate([b_img, b_img]).astype(f32), NLPA)[None, :]
    bf_ = np.tile(np.concatenate([b_fea, b_fea]).astype(f32), NLPA)[None, :]
    bi = bi.astype(bf16)
    bf_ = bf_.astype(bf16)

    def unfold(t):  # (1, 64, 72, 108, 108) -> (C, 10368, 81) patch matrix
        u = np.ascontiguousarray(
            t.reshape(C, D // P, P, HW // P, P).transpose(0, 1, 3, 2, 4)
        ).reshape(C, (D // P) * (HW // P), P2)
        return u

    def pack(u, l0, lhalf):  # global patches [l0, l0+2*lhalf) -> [128, lhalf, 81]
        v = u[:, l0:l0 + 2 * lhalf, :].reshape(C, 2, lhalf, P2)
        v = v.transpose(1, 0, 2, 3).reshape(128, lhalf, P2)
        return np.ascontiguousarray(v).astype(ml_dtypes.bfloat16)

    ux = unfold(np.asarray(x, dtype=f32))
    uy = unfold(np.asarray(y, dtype=f32))
    shared = {"wi": wi, "wf": wf, "wm": wm, "wmy": wmy, "bi": bi, "bf": bf_}
    maps = []
    for k in range(NCORES):
        maps.append(dict(
            shared,
            xp=pack(ux, LX * k, LPX),
            yp=pack(uy, (LY - 128) * k, LPY),
        ))
    return maps


def kernel(x, y, w_img, b_img, w_fea, b_fea, w1, w2):
    if "nc" not in _cache:
        _cache["nc"] = _build_nc()
    nc = _cache["nc"]
    in_maps = _host_prep(x, y, w_img, b_img, w_fea, b_fea, w1, w2)
    trace = bool(os.environ.get("KERNEL_TRACE"))
    res = run_bass_kernel_spmd(
        nc, in_maps, list(range(NCORES)), trace=trace
    )
    _cache["last_result"] = res
    out = np.empty((1, C, D, H, W), dtype=np.float32)
    ov = out.reshape(C, D, HW)
    for k in range(NCORES):
        # out_d is (81, 64, 1296) with l = 648*slot + lp (already global l)
        att = res.results[k]["out"].astype(np.float32).transpose(1, 2, 0)
        blk = att.reshape(C, LX, P, P).transpose(0, 2, 1, 3).reshape(C, P, HW)
        ov[:, P * k:P * (k + 1), :] = blk
    return out
